# revision 21
# baseline (speedup 1.0000x reference)
"""Trainium2 Bass kernel for nn_MultiHeadAttention (B=2,S=2048,D=1024,H=16,HD=64).

Sharding: tensor-parallel over heads (2 heads/core x 8 cores).
Per core:
  Phase A: load pre-transposed X^T (Q/K/V inputs), project to
           Qt/Kt [128(2h*64), 4096] (transposed, bf16) and V [4096, 2x(64+ones)] (bf16).
  Phase B: per (batch, q-chunk): scores^T[keys,q] = Kt_h-tiles @ Qt_h (PE, bf16,
           both heads row-packed for K=64 concurrency), exp via ACT (scale folded
           into Qt; no max-subtraction -- scores are O(1) by construction),
           val^T[65,q] = V_ext^T @ exp_st (ones column -> row 64 = softmax denom),
           val MMs interleaved per double-key-tile to keep PE warm.
  Norm:    per q-chunk: reciprocal_approx_fast on [2,512] denoms, one K=2 PE
           broadcast to [128,512], one in-place DVE multiply into valT.
  Phase C: per-batch AllToAll redistributes val^T (feature-sharded) ->
           concat^T (token-sharded); batch-0 a2a + final-proj overlap batch-1 B.
  Phase D: out^T[o, tok] = WpT-tiles @ concatT-tiles + bp, per batch.
Host: pure layout prep (transposes/slices) + output assembly.

Emission order streams K/V(batch0) first so attention starts ~65us in, keeps
the DMA queues saturated end-to-end (50MB fp32 input reads = the memory floor),
and keeps the PE instruction stream dense (HAM stays at K=8/8).
"""
import sys
import numpy as np

sys.path.insert(0, "/opt/trn_rl_repo")
sys.path.insert(0, "/opt/trn_rl_repo/concourse")

import concourse.bass as bass
import concourse.tile as tile
from concourse import bacc, mybir
from concourse.bass_utils import run_bass_kernel_spmd

FP32 = mybir.dt.float32
BF16 = mybir.dt.bfloat16
AF = mybir.ActivationFunctionType
ALU = mybir.AluOpType

B, S, D, H, HD = 2, 2048, 1024, 16, 64
M = 8                 # cores
HC = H // M           # heads per core
F = HC * HD           # 128 per-core proj features
T = B * S             # 4096 tokens
TS = T // M           # 512 tokens per core (final proj)
SR = S // M           # 256 seq rows per core per batch
SCALE = HD ** -0.5
P = 128
NKT = D // P          # 8 contraction tiles
NST = S // P          # 16 key tiles per batch
NDK = NST // 2        # 8 double key tiles
NQC = S // 512        # 4 q-chunks per batch

_CACHE = {}


def build():
    nc = bacc.Bacc("TRN2", target_bir_lowering=False, debug=False, num_devices=M)

    XT = {x: nc.dram_tensor(f"{x}T", [D, T], FP32, kind="ExternalInput") for x in "qkv"}
    W2 = {x: nc.dram_tensor(f"w2{x}", [D, F], FP32, kind="ExternalInput") for x in "qkv"}
    b2q = nc.dram_tensor("b2q", [F, 1], FP32, kind="ExternalInput")
    b2k = nc.dram_tensor("b2k", [F, 1], FP32, kind="ExternalInput")
    bvb = nc.dram_tensor("bvb", [P, HC * (HD + 1)], FP32, kind="ExternalInput")
    WpT = nc.dram_tensor("WpT", [D, D], FP32, kind="ExternalInput")
    bpT = nc.dram_tensor("bpT", [P, D // P], FP32, kind="ExternalInput")
    onehot_d = nc.dram_tensor("onehot", [HC, P], FP32, kind="ExternalInput")
    outT = nc.dram_tensor("outT", [D, TS], FP32, kind="ExternalOutput")
    a2a_in = [nc.dram_tensor(f"a2a_in{b}", [M, F, SR], BF16) for b in range(B)]
    a2a_out = [nc.dram_tensor(f"a2a_out{b}", [M, F, SR], BF16) for b in range(B)]

    with tile.TileContext(nc) as tc:
        with (
            tc.tile_pool(name="persist", bufs=1) as persist,
            tc.tile_pool(name="xt", bufs=3) as xtp,
            tc.tile_pool(name="xtb", bufs=6) as xtbp,
            tc.tile_pool(name="est", bufs=6) as estp,
            tc.tile_pool(name="small", bufs=4) as small,
            tc.tile_pool(name="den", bufs=2) as denp,
            tc.tile_pool(name="ps_st", bufs=3, space="PSUM") as psA,   # [128,1024] = 2 banks
            tc.tile_pool(name="ps_val", bufs=2, space="PSUM") as psV,  # [128,512] = 1 bank
        ):
            # ---------- constants / weights ----------
            w2b = {}
            for x in "qkv":
                stage = xtp.tile([P, NKT, F], FP32, name="xt")
                nc.sync.dma_start(stage[:], W2[x].ap().rearrange("(kt p) f -> p kt f", p=P))
                w2b[x] = persist.tile([P, NKT, F], BF16, name=f"w2b_{x}")
                nc.vector.tensor_copy(w2b[x][:], stage[:])
            b2q_sb = persist.tile([F, 1], FP32, name="b2q_sb")
            nc.sync.dma_start(b2q_sb[:], b2q[:])
            b2k_sb = persist.tile([F, 1], FP32, name="b2k_sb")
            nc.sync.dma_start(b2k_sb[:], b2k[:])
            bvb_sb = persist.tile([P, HC * (HD + 1)], FP32, name="bvb_sb")
            nc.sync.dma_start(bvb_sb[:], bvb[:])
            bpT_sb = persist.tile([P, D // P], FP32, name="bpT_sb")
            nc.sync.dma_start(bpT_sb[:], bpT[:])
            wpTb = persist.tile([P, NKT, D], BF16, name="wpTb")

            def load_wpT(half):
                stage = xtp.tile([P, NKT, D // 2], FP32, name="xt")
                nc.sync.dma_start(
                    stage[:],
                    WpT.ap().rearrange("(kt p) o -> p kt o", p=P)[:, :, half * 512:(half + 1) * 512],
                )
                nc.vector.tensor_copy(wpTb[:, :, half * 512:(half + 1) * 512], stage[:])
            # head selector: oh2[i, m] = (m // 64 == i); broadcasts [2,512] recips to [128,512]
            onehot2 = persist.tile([HC, P], FP32, name="onehot2")
            nc.sync.dma_start(onehot2[:], onehot_d[:])

            # warmup collective: absorb ncfw first-trigger latency while DMA streams
            warm_in = nc.dram_tensor("cc_warm_in", [M, 1, 64], BF16)
            warm_out = nc.dram_tensor("cc_warm_out", [M, 1, 64], BF16)
            wtile = small.tile([1, M * 64], BF16, name="wtile")
            nc.vector.memset(wtile[:], 1.0)
            nc.gpsimd.dma_start(warm_in.ap().rearrange("j p r -> p (j r)"), wtile[:])
            nc.gpsimd.collective_compute(
                "AllToAll", ALU.bypass, replica_groups=[list(range(M))],
                ins=[warm_in[:]], outs=[warm_out[:]])

            # persistent activations
            Qt = persist.tile([F, T], BF16, name="Qt")        # [2h*64, tok]
            Kt = persist.tile([F, T], BF16, name="Kt")
            Vx = persist.tile([P, T // P, HC * (HD + 1)], BF16, name="Vx")
            valT = persist.tile([F, T], BF16, name="valT")
            nc.vector.memset(Vx[:, :, HD:HD + 1], 1.0)        # ones columns
            nc.vector.memset(Vx[:, :, 2 * HD + 1:2 * HD + 2], 1.0)

            # ---------- phase A pieces (one 512-token chunk of one tensor) ----------
            def load_chunk(x, ch):
                t0 = ch * 512
                stage = xtp.tile([P, NKT, 512], FP32, name="xt")
                nc.sync.dma_start(
                    stage[:],
                    XT[x].ap().rearrange("(kt p) t -> p kt t", p=P)[:, :, t0:t0 + 512],
                )
                xtb = xtbp.tile([P, NKT, 512], BF16, name="xtb")
                nc.vector.tensor_copy(xtb[:], stage[:])
                return xtb

            def proj_qk(x, ch, dest, sc, bias, xtb=None):
                t0 = ch * 512
                if xtb is None:
                    xtb = load_chunk(x, ch)
                ps = psA.tile([P, 1024], FP32, name="ps_st")
                for kt in range(NKT):
                    nc.tensor.matmul(ps[:, 0:512], lhsT=w2b[x][:, kt, :], rhs=xtb[:, kt, :],
                                     start=(kt == 0), stop=(kt == NKT - 1))
                nc.vector.tensor_scalar(dest[:, t0:t0 + 512], ps[:, 0:512], sc, bias[:, 0:1],
                                        op0=ALU.mult, op1=ALU.add)

            def proj_v(ch, xtb=None):
                if xtb is None:
                    xtb = load_chunk("v", ch)
                vps = psA.tile([P, 1024], FP32, name="ps_st")
                for sub in range(4):
                    for kt in range(NKT):
                        nc.tensor.matmul(vps[:, sub * F:(sub + 1) * F],
                                         lhsT=xtb[:, kt, sub * P:(sub + 1) * P],
                                         rhs=w2b["v"][:, kt, :],
                                         start=(kt == 0), stop=(kt == NKT - 1))
                for sub in range(4):
                    tt = ch * 4 + sub
                    for h in range(HC):
                        nc.vector.tensor_add(Vx[:, tt, h * 65:h * 65 + HD],
                                             vps[:, sub * F + h * HD:sub * F + (h + 1) * HD],
                                             bvb_sb[:, h * 65:h * 65 + HD])

            # ---------- Phase B + incremental normalization ----------
            def phase_b(b, qc, extra_work=None):
                q0 = b * S + qc * 512
                vps = [psV.tile([P, 512], FP32, name="ps_val") for _ in range(HC)]
                est_prev = None
                for dk in range(NDK):
                    if extra_work is not None and dk in extra_work:
                        extra_work[dk]()
                    k0 = b * S + dk * 256
                    est = []
                    for h in range(HC):
                        fo = h * HD
                        stp = psA.tile([P, 1024], FP32, name="ps_st")
                        for half in range(2):
                            nc.tensor.matmul(stp[:, half * 512:(half + 1) * 512],
                                             lhsT=Kt[fo:fo + HD, k0 + half * P:k0 + (half + 1) * P],
                                             rhs=Qt[fo:fo + HD, q0:q0 + 512],
                                             start=True, stop=True)
                        e = estp.tile([P, 1024], BF16, name="est")
                        nc.scalar.activation(e[:], stp[:], AF.Exp)
                        est.append(e)
                    # val MMs for the previous double-tile (keeps PE fed while ACT runs)
                    if est_prev is not None:
                        emit_val(b, qc, dk - 1, est_prev, vps)
                    est_prev = est
                emit_val(b, qc, NDK - 1, est_prev, vps)
                # denominators -> den tile rows [2, 512]
                den_t = denp.tile([HC, 512], FP32, name="den")
                for h in range(HC):
                    dstage = small.tile([P, 512], FP32, name="dstage")
                    nc.vector.tensor_copy(dstage[HD:HD + 1, :], vps[h][HD:HD + 1, :])
                    nc.gpsimd.dma_start(den_t[h:h + 1, :], dstage[HD:HD + 1, :])
                    # unnormalized val^T -> valT (bf16)
                    nc.vector.tensor_copy(valT[h * HD:(h + 1) * HD, q0:q0 + 512],
                                          vps[h][0:HD, :])
                rcp_t = denp.tile([HC, 512], FP32, name="rcp")
                nc.vector.reciprocal_approx_fast(rcp_t[:], den_t[:])
                rbp = psV.tile([P, 512], FP32, name="ps_val")
                nc.tensor.matmul(rbp[:], lhsT=onehot2[:], rhs=rcp_t[:], start=True, stop=True)
                nc.vector.tensor_mul(valT[:, q0:q0 + 512], rbp[:], valT[:, q0:q0 + 512])

            def emit_val(b, qc, dk, est, vps):
                for h in range(HC):
                    for half in range(2):
                        kt = dk * 2 + half
                        nc.tensor.matmul(vps[h][0:HD + 1, :],
                                         lhsT=Vx[:, b * NST + kt, h * 65:(h + 1) * 65],
                                         rhs=est[h][:, half * 512:(half + 1) * 512],
                                         start=(kt == 0), stop=(kt == NST - 1))

            def a2a_send(b):
                nc.gpsimd.dma_start(
                    a2a_in[b].ap().rearrange("j p r -> p j r"),
                    valT.rearrange("p (bb j r) -> p bb j r", bb=B, j=M, r=SR)[:, b, :, :],
                )
                nc.gpsimd.collective_compute(
                    "AllToAll", ALU.bypass,
                    replica_groups=[list(range(M))],
                    ins=[a2a_in[b][:]], outs=[a2a_out[b][:]],
                )

            def phase_d_og(b, concatT, og):
                    ops = psA.tile([P, 1024], FP32, name="ps_st")
                    for sub in range(4):
                        oc = og * 4 + sub
                        for ft in range(NKT):
                            nc.tensor.matmul(ops[:, sub * SR:(sub + 1) * SR],
                                             lhsT=wpTb[:, ft, oc * P:(oc + 1) * P],
                                             rhs=concatT[:, ft, :],
                                             start=(ft == 0), stop=(ft == NKT - 1))
                    ot = small.tile([P, 1024], FP32, name="ot")
                    for sub in range(4):
                        oc = og * 4 + sub
                        nc.vector.tensor_scalar_add(ot[:, sub * SR:(sub + 1) * SR],
                                                    ops[:, sub * SR:(sub + 1) * SR],
                                                    bpT_sb[:, oc:oc + 1])
                    nc.gpsimd.dma_start(
                        outT.ap().rearrange("(og oc p) (bb r) -> p og oc bb r", p=P, oc=4, bb=B)[:, og, :, b, :],
                        ot.rearrange("p (oc r) -> p oc r", oc=4))

            def phase_d(b, concatT):
                nc.sync.dma_start(concatT[:], a2a_out[b].ap().rearrange("j p r -> p j r"))
                for og in range(2):
                    ops = psA.tile([P, 1024], FP32, name="ps_st")
                    for sub in range(4):
                        oc = og * 4 + sub
                        for ft in range(NKT):
                            nc.tensor.matmul(ops[:, sub * SR:(sub + 1) * SR],
                                             lhsT=wpTb[:, ft, oc * P:(oc + 1) * P],
                                             rhs=concatT[:, ft, :],
                                             start=(ft == 0), stop=(ft == NKT - 1))
                    ot = small.tile([P, 1024], FP32, name="ot")
                    for sub in range(4):
                        oc = og * 4 + sub
                        nc.vector.tensor_scalar_add(ot[:, sub * SR:(sub + 1) * SR],
                                                    ops[:, sub * SR:(sub + 1) * SR],
                                                    bpT_sb[:, oc:oc + 1])
                    nc.gpsimd.dma_start(
                        outT.ap().rearrange("(og oc p) (bb r) -> p og oc bb r", p=P, oc=4, bb=B)[:, og, :, b, :],
                        ot.rearrange("p (oc r) -> p oc r", oc=4))

            # ---------- emission ----------
            # Attention starts after only k0+v0+q0; remaining projections are
            # interleaved into phase-B chunks as extra work, with their loads
            # pre-issued in consumption order so PE never stalls on a cast.
            concatT0 = persist.tile([P, M, SR], BF16, name="concatT0")
            concatT1 = persist.tile([P, M, SR], BF16, name="concatT1")
            pre = {}

            def pk(x, ch):
                pre[f"{x}{ch}"] = load_chunk(x, ch)

            def pj(x, ch):
                if x == "v":
                    proj_v(ch, xtb=pre[f"v{ch}"])
                elif x == "k":
                    proj_qk("k", ch, Kt, 1.0, b2k_sb, xtb=pre[f"k{ch}"])
                else:
                    proj_qk("q", ch, Qt, SCALE, b2q_sb, xtb=pre[f"q{ch}"])

            proj_qk("k", 0, Kt, 1.0, b2k_sb)
            proj_v(0)
            proj_qk("q", 0, Qt, SCALE, b2q_sb)
            pk("k", 1); pk("v", 1); pk("k", 2); pk("v", 2)
            phase_b(0, 0, extra_work={
                2: lambda: (pj("k", 1), pj("v", 1), pk("k", 3), pk("v", 3)),
                4: lambda: (pj("k", 2), pj("v", 2), pk("q", 1), pk("k", 4)),
                6: lambda: (pj("k", 3), pj("v", 3), pk("v", 4)),
            })
            pj("q", 1)
            phase_b(0, 1, extra_work={
                3: lambda: (pj("k", 4), pj("v", 4), pk("q", 2), pk("k", 5), pk("v", 5)),
            })
            load_wpT(0)
            pj("q", 2)
            phase_b(0, 2, extra_work={
                3: lambda: (pj("k", 5), pj("v", 5), pk("q", 3), pk("k", 6), pk("v", 6)),
            })
            load_wpT(1)
            pj("q", 3)
            phase_b(0, 3, extra_work={
                2: lambda: (pj("k", 6), pj("v", 6), pk("k", 7), pk("v", 7)),
                5: lambda: (pj("k", 7), pj("v", 7), pk("q", 4)),
            })
            a2a_send(0)
            pj("q", 4)
            pk("q", 5)
            phase_b(1, 0)
            pj("q", 5)
            pk("q", 6)
            phase_b(1, 1)
            nc.sync.dma_start(concatT0[:], a2a_out[0].ap().rearrange("j p r -> p j r"))
            pj("q", 6)
            pk("q", 7)
            phase_b(1, 2)
            phase_d_og(0, concatT0, 0)
            pj("q", 7)
            phase_b(1, 3)
            a2a_send(1)
            phase_d_og(0, concatT0, 1)
            # idempotent re-issue: keeps PE busy through the a2a(1) flight so
            # HAM stays warm for phase D(1) (rewrites identical outT bytes)
            phase_d_og(0, concatT0, 0)
            nc.sync.dma_start(concatT1[:], a2a_out[1].ap().rearrange("j p r -> p j r"))
            phase_d_og(1, concatT1, 0)
            phase_d_og(1, concatT1, 1)

    nc.compile()
    return nc


def _host_prep(inputs):
    f32 = np.float32
    QT = np.ascontiguousarray(inputs["Q_in"].reshape(T, D).T).astype(f32, copy=False)
    KT = np.ascontiguousarray(inputs["K_in"].reshape(T, D).T).astype(f32, copy=False)
    VT = np.ascontiguousarray(inputs["V_in"].reshape(T, D).T).astype(f32, copy=False)
    WpT = np.ascontiguousarray(inputs["Wp"].T).astype(f32, copy=False)
    bpT = np.ascontiguousarray(inputs["bp"].reshape(D // P, P).T).astype(f32, copy=False)
    oh2 = np.zeros((HC, P), f32)
    for h in range(HC):
        oh2[h, h * HD:(h + 1) * HD] = 1.0
    in_maps = []
    for c in range(M):
        sl = slice(c * HC, (c + 1) * HC)
        m = {
            "qT": QT, "kT": KT, "vT": VT, "WpT": WpT, "bpT": bpT, "onehot": oh2,
            "w2q": np.ascontiguousarray(inputs["Wq"][sl].transpose(1, 0, 2).reshape(D, F)).astype(f32, copy=False),
            "w2k": np.ascontiguousarray(inputs["Wk"][sl].transpose(1, 0, 2).reshape(D, F)).astype(f32, copy=False),
            "w2v": np.ascontiguousarray(inputs["Wv"][sl].transpose(1, 0, 2).reshape(D, F)).astype(f32, copy=False),
            "b2q": (inputs["bq"][sl].reshape(F, 1) * SCALE).astype(f32),
            "b2k": inputs["bk"][sl].reshape(F, 1).astype(f32),
        }
        bvb = np.zeros((P, HC * (HD + 1)), f32)
        for h in range(HC):
            bvb[:, h * 65:h * 65 + HD] = inputs["bv"][c * HC + h][None, :]
        m["bvb"] = bvb
        in_maps.append(m)
    return in_maps


_LAST = {"exec_time_ns": None}


def kernel(**inputs):
    inputs = {k: np.asarray(v) for k, v in inputs.items()}
    if "nc" not in _CACHE:
        _CACHE["nc"] = build()
    nc = _CACHE["nc"]
    in_maps = _host_prep(inputs)
    res = run_bass_kernel_spmd(nc, in_maps, core_ids=list(range(M)),
                               trace=_LAST.get("trace", False))
    _LAST["exec_time_ns"] = res.exec_time_ns
    _LAST["res"] = res
    out = np.zeros((T, D), np.float32)
    for c in range(M):
        oT = res.results[c]["outT"]  # [D, TS] = [D, (b sr)]
        for b in range(B):
            out[b * S + c * SR:b * S + (c + 1) * SR, :] = oT[:, b * SR:(b + 1) * SR].T
    return out.reshape(B, S, D)


# revision 22
# speedup vs baseline: 1.0041x; 1.0041x over previous
"""Trainium2 Bass kernel for nn_MultiHeadAttention (B=2,S=2048,D=1024,H=16,HD=64).

Sharding: tensor-parallel over heads (2 heads/core x 8 cores).
Per core:
  Phase A: load pre-transposed X^T (Q/K/V inputs), project to
           Qt/Kt [128(2h*64), 4096] (transposed, bf16) and V [4096, 2x(64+ones)] (bf16).
  Phase B: per (batch, q-chunk): scores^T[keys,q] = Kt_h-tiles @ Qt_h (PE, bf16,
           both heads row-packed for K=64 concurrency), exp via ACT (scale folded
           into Qt; no max-subtraction -- scores are O(1) by construction),
           val^T[65,q] = V_ext^T @ exp_st (ones column -> row 64 = softmax denom),
           val MMs interleaved per double-key-tile to keep PE warm.
  Norm:    per q-chunk: reciprocal_approx_fast on [2,512] denoms, one K=2 PE
           broadcast to [128,512], one in-place DVE multiply into valT.
  Phase C: per-batch AllToAll redistributes val^T (feature-sharded) ->
           concat^T (token-sharded); batch-0 a2a + final-proj overlap batch-1 B.
  Phase D: out^T[o, tok] = WpT-tiles @ concatT-tiles + bp, per batch.
Host: pure layout prep (transposes/slices) + output assembly.

Emission order streams K/V(batch0) first so attention starts ~65us in, keeps
the DMA queues saturated end-to-end (50MB fp32 input reads = the memory floor),
and keeps the PE instruction stream dense (HAM stays at K=8/8).
"""
import sys
import numpy as np

sys.path.insert(0, "/opt/trn_rl_repo")
sys.path.insert(0, "/opt/trn_rl_repo/concourse")

import concourse.bass as bass
import concourse.tile as tile
from concourse import bacc, mybir
from concourse.bass_utils import run_bass_kernel_spmd

FP32 = mybir.dt.float32
BF16 = mybir.dt.bfloat16
AF = mybir.ActivationFunctionType
ALU = mybir.AluOpType

B, S, D, H, HD = 2, 2048, 1024, 16, 64
M = 8                 # cores
HC = H // M           # heads per core
F = HC * HD           # 128 per-core proj features
T = B * S             # 4096 tokens
TS = T // M           # 512 tokens per core (final proj)
SR = S // M           # 256 seq rows per core per batch
SCALE = HD ** -0.5
P = 128
NKT = D // P          # 8 contraction tiles
NST = S // P          # 16 key tiles per batch
NDK = NST // 2        # 8 double key tiles
NQC = S // 512        # 4 q-chunks per batch

_CACHE = {}


def build():
    nc = bacc.Bacc("TRN2", target_bir_lowering=False, debug=False, num_devices=M)

    XT = {x: nc.dram_tensor(f"{x}T", [D, T], FP32, kind="ExternalInput") for x in "qkv"}
    W2 = {x: nc.dram_tensor(f"w2{x}", [D, F], FP32, kind="ExternalInput") for x in "qkv"}
    b2q = nc.dram_tensor("b2q", [F, 1], FP32, kind="ExternalInput")
    b2k = nc.dram_tensor("b2k", [F, 1], FP32, kind="ExternalInput")
    bvb = nc.dram_tensor("bvb", [P, HC * (HD + 1)], FP32, kind="ExternalInput")
    WpT = nc.dram_tensor("WpT", [D, D], FP32, kind="ExternalInput")
    bpT = nc.dram_tensor("bpT", [P, D // P], FP32, kind="ExternalInput")
    onehot_d = nc.dram_tensor("onehot", [HC, P], FP32, kind="ExternalInput")
    outT = nc.dram_tensor("outT", [D, TS], FP32, kind="ExternalOutput")
    a2a_in = [nc.dram_tensor(f"a2a_in{b}", [M, F, SR], BF16) for b in range(B)]
    a2a_out = [nc.dram_tensor(f"a2a_out{b}", [M, F, SR], BF16) for b in range(B)]

    with tile.TileContext(nc) as tc:
        with (
            tc.tile_pool(name="persist", bufs=1) as persist,
            tc.tile_pool(name="xt", bufs=4) as xtp,
            tc.tile_pool(name="xtb", bufs=3) as xtbp,
            tc.tile_pool(name="est", bufs=6) as estp,
            tc.tile_pool(name="small", bufs=4) as small,
            tc.tile_pool(name="den", bufs=4) as denp,
            tc.tile_pool(name="ps_st", bufs=3, space="PSUM") as psA,   # [128,1024] = 2 banks
            tc.tile_pool(name="ps_val", bufs=2, space="PSUM") as psV,  # [128,512] = 1 bank
        ):
            # ---------- constants / weights ----------
            w2b = {}
            for x in "qkv":
                stage = xtp.tile([P, NKT, F], FP32, name="xt")
                nc.sync.dma_start(stage[:], W2[x].ap().rearrange("(kt p) f -> p kt f", p=P))
                w2b[x] = persist.tile([P, NKT, F], BF16, name=f"w2b_{x}")
                nc.vector.tensor_copy(w2b[x][:], stage[:])
            b2q_sb = persist.tile([F, 1], FP32, name="b2q_sb")
            nc.sync.dma_start(b2q_sb[:], b2q[:])
            b2k_sb = persist.tile([F, 1], FP32, name="b2k_sb")
            nc.sync.dma_start(b2k_sb[:], b2k[:])
            bvb_sb = persist.tile([P, HC * (HD + 1)], FP32, name="bvb_sb")
            nc.sync.dma_start(bvb_sb[:], bvb[:])
            bpT_sb = persist.tile([P, D // P], FP32, name="bpT_sb")
            nc.sync.dma_start(bpT_sb[:], bpT[:])
            wpTb = persist.tile([P, NKT, D], BF16, name="wpTb")

            def load_wpT(half):
                stage = xtp.tile([P, NKT, D // 2], FP32, name="xt")
                nc.sync.dma_start(
                    stage[:],
                    WpT.ap().rearrange("(kt p) o -> p kt o", p=P)[:, :, half * 512:(half + 1) * 512],
                )
                nc.vector.tensor_copy(wpTb[:, :, half * 512:(half + 1) * 512], stage[:])
            # head selector: oh2[i, m] = (m // 64 == i); broadcasts [2,512] recips to [128,512]
            onehot2 = persist.tile([HC, P], FP32, name="onehot2")
            nc.sync.dma_start(onehot2[:], onehot_d[:])

            # warmup collective: absorb ncfw first-trigger latency while DMA streams
            warm_in = nc.dram_tensor("cc_warm_in", [M, 1, 64], BF16)
            warm_out = nc.dram_tensor("cc_warm_out", [M, 1, 64], BF16)
            wtile = small.tile([1, M * 64], BF16, name="wtile")
            nc.vector.memset(wtile[:], 1.0)
            nc.gpsimd.dma_start(warm_in.ap().rearrange("j p r -> p (j r)"), wtile[:])
            nc.gpsimd.collective_compute(
                "AllToAll", ALU.bypass, replica_groups=[list(range(M))],
                ins=[warm_in[:]], outs=[warm_out[:]])

            # persistent activations
            Qt = persist.tile([F, T], BF16, name="Qt")        # [2h*64, tok]
            Kt = persist.tile([F, T], BF16, name="Kt")
            Vx = persist.tile([P, T // P, HC * (HD + 1)], BF16, name="Vx")
            valT = persist.tile([F, T], BF16, name="valT")
            nc.vector.memset(Vx[:, :, HD:HD + 1], 1.0)        # ones columns
            nc.vector.memset(Vx[:, :, 2 * HD + 1:2 * HD + 2], 1.0)

            # ---------- phase A pieces (one 512-token chunk of one tensor) ----------
            def load_chunk(x, ch):
                t0 = ch * 512
                stage = xtp.tile([P, NKT, 512], FP32, name="xt")
                nc.sync.dma_start(
                    stage[:],
                    XT[x].ap().rearrange("(kt p) t -> p kt t", p=P)[:, :, t0:t0 + 512],
                )
                xtb = xtbp.tile([P, NKT, 512], BF16, name="xtb")
                nc.vector.tensor_copy(xtb[:], stage[:])
                return xtb

            def proj_qk(x, ch, dest, sc, bias, xtb=None):
                t0 = ch * 512
                if xtb is None:
                    xtb = load_chunk(x, ch)
                ps = psA.tile([P, 1024], FP32, name="ps_st")
                for kt in range(NKT):
                    nc.tensor.matmul(ps[:, 0:512], lhsT=w2b[x][:, kt, :], rhs=xtb[:, kt, :],
                                     start=(kt == 0), stop=(kt == NKT - 1))
                nc.vector.tensor_scalar(dest[:, t0:t0 + 512], ps[:, 0:512], sc, bias[:, 0:1],
                                        op0=ALU.mult, op1=ALU.add)

            def proj_v(ch, xtb=None):
                if xtb is None:
                    xtb = load_chunk("v", ch)
                vps = psA.tile([P, 1024], FP32, name="ps_st")
                for sub in range(4):
                    for kt in range(NKT):
                        nc.tensor.matmul(vps[:, sub * F:(sub + 1) * F],
                                         lhsT=xtb[:, kt, sub * P:(sub + 1) * P],
                                         rhs=w2b["v"][:, kt, :],
                                         start=(kt == 0), stop=(kt == NKT - 1))
                for sub in range(4):
                    tt = ch * 4 + sub
                    for h in range(HC):
                        nc.vector.tensor_add(Vx[:, tt, h * 65:h * 65 + HD],
                                             vps[:, sub * F + h * HD:sub * F + (h + 1) * HD],
                                             bvb_sb[:, h * 65:h * 65 + HD])

            # ---------- Phase B + incremental normalization ----------
            def phase_b(b, qc, extra_work=None):
                q0 = b * S + qc * 512
                vps = [psV.tile([P, 512], FP32, name="ps_val") for _ in range(HC)]
                est_prev = None
                for dk in range(NDK):
                    if extra_work is not None and dk in extra_work:
                        extra_work[dk]()
                    k0 = b * S + dk * 256
                    est = []
                    for h in range(HC):
                        fo = h * HD
                        stp = psA.tile([P, 1024], FP32, name="ps_st")
                        for half in range(2):
                            nc.tensor.matmul(stp[:, half * 512:(half + 1) * 512],
                                             lhsT=Kt[fo:fo + HD, k0 + half * P:k0 + (half + 1) * P],
                                             rhs=Qt[fo:fo + HD, q0:q0 + 512],
                                             start=True, stop=True)
                        e = estp.tile([P, 1024], BF16, name="est")
                        nc.scalar.activation(e[:], stp[:], AF.Exp)
                        est.append(e)
                    # val MMs for the previous double-tile (keeps PE fed while ACT runs)
                    if est_prev is not None:
                        emit_val(b, qc, dk - 1, est_prev, vps)
                    est_prev = est
                emit_val(b, qc, NDK - 1, est_prev, vps)
                # denominators -> den tile rows [2, 512]
                den_t = denp.tile([HC, 512], FP32, name="den")
                for h in range(HC):
                    dstage = small.tile([P, 512], FP32, name="dstage")
                    nc.vector.tensor_copy(dstage[HD:HD + 1, :], vps[h][HD:HD + 1, :])
                    nc.gpsimd.dma_start(den_t[h:h + 1, :], dstage[HD:HD + 1, :])
                    # unnormalized val^T -> valT (bf16)
                    nc.vector.tensor_copy(valT[h * HD:(h + 1) * HD, q0:q0 + 512],
                                          vps[h][0:HD, :])
                rcp_t = denp.tile([HC, 512], FP32, name="rcp")
                nc.vector.reciprocal_approx_fast(rcp_t[:], den_t[:])
                rbp = psV.tile([P, 512], FP32, name="ps_val")
                nc.tensor.matmul(rbp[:], lhsT=onehot2[:], rhs=rcp_t[:], start=True, stop=True)
                nc.vector.tensor_mul(valT[:, q0:q0 + 512], rbp[:], valT[:, q0:q0 + 512])

            def emit_val(b, qc, dk, est, vps):
                for h in range(HC):
                    for half in range(2):
                        kt = dk * 2 + half
                        nc.tensor.matmul(vps[h][0:HD + 1, :],
                                         lhsT=Vx[:, b * NST + kt, h * 65:(h + 1) * 65],
                                         rhs=est[h][:, half * 512:(half + 1) * 512],
                                         start=(kt == 0), stop=(kt == NST - 1))

            def a2a_send(b):
                nc.gpsimd.dma_start(
                    a2a_in[b].ap().rearrange("j p r -> p j r"),
                    valT.rearrange("p (bb j r) -> p bb j r", bb=B, j=M, r=SR)[:, b, :, :],
                )
                nc.gpsimd.collective_compute(
                    "AllToAll", ALU.bypass,
                    replica_groups=[list(range(M))],
                    ins=[a2a_in[b][:]], outs=[a2a_out[b][:]],
                )

            def phase_d_og(b, concatT, og):
                    ops = psA.tile([P, 1024], FP32, name="ps_st")
                    for sub in range(4):
                        oc = og * 4 + sub
                        for ft in range(NKT):
                            nc.tensor.matmul(ops[:, sub * SR:(sub + 1) * SR],
                                             lhsT=wpTb[:, ft, oc * P:(oc + 1) * P],
                                             rhs=concatT[:, ft, :],
                                             start=(ft == 0), stop=(ft == NKT - 1))
                    ot = small.tile([P, 1024], FP32, name="ot")
                    for sub in range(4):
                        oc = og * 4 + sub
                        nc.vector.tensor_scalar_add(ot[:, sub * SR:(sub + 1) * SR],
                                                    ops[:, sub * SR:(sub + 1) * SR],
                                                    bpT_sb[:, oc:oc + 1])
                    nc.gpsimd.dma_start(
                        outT.ap().rearrange("(og oc p) (bb r) -> p og oc bb r", p=P, oc=4, bb=B)[:, og, :, b, :],
                        ot.rearrange("p (oc r) -> p oc r", oc=4))

            def phase_d(b, concatT):
                nc.sync.dma_start(concatT[:], a2a_out[b].ap().rearrange("j p r -> p j r"))
                for og in range(2):
                    ops = psA.tile([P, 1024], FP32, name="ps_st")
                    for sub in range(4):
                        oc = og * 4 + sub
                        for ft in range(NKT):
                            nc.tensor.matmul(ops[:, sub * SR:(sub + 1) * SR],
                                             lhsT=wpTb[:, ft, oc * P:(oc + 1) * P],
                                             rhs=concatT[:, ft, :],
                                             start=(ft == 0), stop=(ft == NKT - 1))
                    ot = small.tile([P, 1024], FP32, name="ot")
                    for sub in range(4):
                        oc = og * 4 + sub
                        nc.vector.tensor_scalar_add(ot[:, sub * SR:(sub + 1) * SR],
                                                    ops[:, sub * SR:(sub + 1) * SR],
                                                    bpT_sb[:, oc:oc + 1])
                    nc.gpsimd.dma_start(
                        outT.ap().rearrange("(og oc p) (bb r) -> p og oc bb r", p=P, oc=4, bb=B)[:, og, :, b, :],
                        ot.rearrange("p (oc r) -> p oc r", oc=4))

            # ---------- emission ----------
            # Attention starts after only k0+v0+q0; remaining projections are
            # interleaved into phase-B chunks as extra work, with their loads
            # pre-issued in consumption order so PE never stalls on a cast.
            concatT0 = persist.tile([P, M, SR], BF16, name="concatT0")
            concatT1 = persist.tile([P, M, SR], BF16, name="concatT1")
            pre = {}

            def pk(x, ch):
                pre[f"{x}{ch}"] = load_chunk(x, ch)

            def pj(x, ch):
                if x == "v":
                    proj_v(ch, xtb=pre[f"v{ch}"])
                elif x == "k":
                    proj_qk("k", ch, Kt, 1.0, b2k_sb, xtb=pre[f"k{ch}"])
                else:
                    proj_qk("q", ch, Qt, SCALE, b2q_sb, xtb=pre[f"q{ch}"])

            proj_qk("k", 0, Kt, 1.0, b2k_sb)
            proj_v(0)
            proj_qk("q", 0, Qt, SCALE, b2q_sb)
            pk("k", 1); pk("v", 1); pk("k", 2); pk("v", 2)
            phase_b(0, 0, extra_work={
                2: lambda: (pj("k", 1), pj("v", 1), pk("k", 3), pk("v", 3)),
                4: lambda: (pj("k", 2), pj("v", 2), pk("q", 1), pk("k", 4)),
                6: lambda: (pj("k", 3), pj("v", 3), pk("v", 4)),
            })
            pj("q", 1)
            phase_b(0, 1, extra_work={
                3: lambda: (pj("k", 4), pj("v", 4), pk("q", 2), pk("k", 5), pk("v", 5)),
            })
            load_wpT(0)
            pj("q", 2)
            phase_b(0, 2, extra_work={
                3: lambda: (pj("k", 5), pj("v", 5), pk("q", 3), pk("k", 6), pk("v", 6)),
            })
            load_wpT(1)
            pj("q", 3)
            phase_b(0, 3, extra_work={
                2: lambda: (pj("k", 6), pj("v", 6), pk("k", 7), pk("v", 7)),
                5: lambda: (pj("k", 7), pj("v", 7), pk("q", 4)),
            })
            a2a_send(0)
            pj("q", 4)
            pk("q", 5)
            phase_b(1, 0)
            pj("q", 5)
            pk("q", 6)
            phase_b(1, 1)
            nc.sync.dma_start(concatT0[:], a2a_out[0].ap().rearrange("j p r -> p j r"))
            pj("q", 6)
            pk("q", 7)
            phase_b(1, 2)
            phase_d_og(0, concatT0, 0)
            pj("q", 7)
            phase_b(1, 3)
            a2a_send(1)
            phase_d_og(0, concatT0, 1)
            # idempotent re-issue: keeps PE busy through the a2a(1) flight so
            # HAM stays warm for phase D(1) (rewrites identical outT bytes)
            phase_d_og(0, concatT0, 0)
            nc.sync.dma_start(concatT1[:], a2a_out[1].ap().rearrange("j p r -> p j r"))
            phase_d_og(1, concatT1, 0)
            phase_d_og(1, concatT1, 1)

    nc.compile()
    return nc


def _host_prep(inputs):
    f32 = np.float32
    QT = np.ascontiguousarray(inputs["Q_in"].reshape(T, D).T).astype(f32, copy=False)
    KT = np.ascontiguousarray(inputs["K_in"].reshape(T, D).T).astype(f32, copy=False)
    VT = np.ascontiguousarray(inputs["V_in"].reshape(T, D).T).astype(f32, copy=False)
    WpT = np.ascontiguousarray(inputs["Wp"].T).astype(f32, copy=False)
    bpT = np.ascontiguousarray(inputs["bp"].reshape(D // P, P).T).astype(f32, copy=False)
    oh2 = np.zeros((HC, P), f32)
    for h in range(HC):
        oh2[h, h * HD:(h + 1) * HD] = 1.0
    in_maps = []
    for c in range(M):
        sl = slice(c * HC, (c + 1) * HC)
        m = {
            "qT": QT, "kT": KT, "vT": VT, "WpT": WpT, "bpT": bpT, "onehot": oh2,
            "w2q": np.ascontiguousarray(inputs["Wq"][sl].transpose(1, 0, 2).reshape(D, F)).astype(f32, copy=False),
            "w2k": np.ascontiguousarray(inputs["Wk"][sl].transpose(1, 0, 2).reshape(D, F)).astype(f32, copy=False),
            "w2v": np.ascontiguousarray(inputs["Wv"][sl].transpose(1, 0, 2).reshape(D, F)).astype(f32, copy=False),
            "b2q": (inputs["bq"][sl].reshape(F, 1) * SCALE).astype(f32),
            "b2k": inputs["bk"][sl].reshape(F, 1).astype(f32),
        }
        bvb = np.zeros((P, HC * (HD + 1)), f32)
        for h in range(HC):
            bvb[:, h * 65:h * 65 + HD] = inputs["bv"][c * HC + h][None, :]
        m["bvb"] = bvb
        in_maps.append(m)
    return in_maps


_LAST = {"exec_time_ns": None}


def kernel(**inputs):
    inputs = {k: np.asarray(v) for k, v in inputs.items()}
    if "nc" not in _CACHE:
        _CACHE["nc"] = build()
    nc = _CACHE["nc"]
    in_maps = _host_prep(inputs)
    res = run_bass_kernel_spmd(nc, in_maps, core_ids=list(range(M)),
                               trace=_LAST.get("trace", False))
    _LAST["exec_time_ns"] = res.exec_time_ns
    _LAST["res"] = res
    out = np.zeros((T, D), np.float32)
    for c in range(M):
        oT = res.results[c]["outT"]  # [D, TS] = [D, (b sr)]
        for b in range(B):
            out[b * S + c * SR:b * S + (c + 1) * SR, :] = oT[:, b * SR:(b + 1) * SR].T
    return out.reshape(B, S, D)


# revision 23
# speedup vs baseline: 1.0123x; 1.0081x over previous
"""Trainium2 Bass kernel for nn_MultiHeadAttention (B=2,S=2048,D=1024,H=16,HD=64).

Sharding: tensor-parallel over heads (2 heads/core x 8 cores).
Per core:
  Phase A: load pre-transposed X^T (Q/K/V inputs), project to
           Qt/Kt [128(2h*64), 4096] (transposed, bf16) and V [4096, 2x(64+ones)] (bf16).
  Phase B: per (batch, q-chunk): scores^T[keys,q] = Kt_h-tiles @ Qt_h (PE, bf16,
           both heads row-packed for K=64 concurrency), exp via ACT (scale folded
           into Qt; no max-subtraction -- scores are O(1) by construction),
           val^T[65,q] = V_ext^T @ exp_st (ones column -> row 64 = softmax denom),
           val MMs interleaved per double-key-tile to keep PE warm.
  Norm:    per q-chunk: reciprocal_approx_fast on [2,512] denoms, one K=2 PE
           broadcast to [128,512], one in-place DVE multiply into valT.
  Phase C: per-batch AllToAll redistributes val^T (feature-sharded) ->
           concat^T (token-sharded); batch-0 a2a + final-proj overlap batch-1 B.
  Phase D: out^T[o, tok] = WpT-tiles @ concatT-tiles + bp, per batch.
Host: pure layout prep (transposes/slices) + output assembly.

Emission order streams K/V(batch0) first so attention starts ~65us in, keeps
the DMA queues saturated end-to-end (50MB fp32 input reads = the memory floor),
and keeps the PE instruction stream dense (HAM stays at K=8/8).
"""
import sys
import numpy as np

sys.path.insert(0, "/opt/trn_rl_repo")
sys.path.insert(0, "/opt/trn_rl_repo/concourse")

import concourse.bass as bass
import concourse.tile as tile
from concourse import bacc, mybir
from concourse.bass_utils import run_bass_kernel_spmd

FP32 = mybir.dt.float32
BF16 = mybir.dt.bfloat16
AF = mybir.ActivationFunctionType
ALU = mybir.AluOpType

B, S, D, H, HD = 2, 2048, 1024, 16, 64
M = 8                 # cores
HC = H // M           # heads per core
F = HC * HD           # 128 per-core proj features
T = B * S             # 4096 tokens
TS = T // M           # 512 tokens per core (final proj)
SR = S // M           # 256 seq rows per core per batch
SCALE = HD ** -0.5
P = 128
NKT = D // P          # 8 contraction tiles
NST = S // P          # 16 key tiles per batch
NDK = NST // 2        # 8 double key tiles
NQC = S // 512        # 4 q-chunks per batch

_CACHE = {}


def build():
    nc = bacc.Bacc("TRN2", target_bir_lowering=False, debug=False, num_devices=M)

    XT = {x: nc.dram_tensor(f"{x}T", [D, T], FP32, kind="ExternalInput") for x in "qkv"}
    W2 = {x: nc.dram_tensor(f"w2{x}", [D, F], FP32, kind="ExternalInput") for x in "qkv"}
    b2q = nc.dram_tensor("b2q", [F, 1], FP32, kind="ExternalInput")
    b2k = nc.dram_tensor("b2k", [F, 1], FP32, kind="ExternalInput")
    bvb = nc.dram_tensor("bvb", [P, HC * (HD + 1)], FP32, kind="ExternalInput")
    WpT = nc.dram_tensor("WpT", [D, D], FP32, kind="ExternalInput")
    bpT = nc.dram_tensor("bpT", [P, D // P], FP32, kind="ExternalInput")
    onehot_d = nc.dram_tensor("onehot", [HC, P], FP32, kind="ExternalInput")
    outT = nc.dram_tensor("outT", [D, TS], FP32, kind="ExternalOutput")
    a2a_in = [nc.dram_tensor(f"a2a_in{b}", [M, F, SR], BF16) for b in range(B)]
    a2a_out = [nc.dram_tensor(f"a2a_out{b}", [M, F, SR], BF16) for b in range(B)]

    with tile.TileContext(nc) as tc:
        with (
            tc.tile_pool(name="persist", bufs=1) as persist,
            tc.tile_pool(name="xt", bufs=4) as xtp,
            tc.tile_pool(name="xtb", bufs=3) as xtbp,
            tc.tile_pool(name="est", bufs=6) as estp,
            tc.tile_pool(name="small", bufs=4) as small,
            tc.tile_pool(name="den", bufs=4) as denp,
            tc.tile_pool(name="ps_st", bufs=3, space="PSUM") as psA,   # [128,1024] = 2 banks
            tc.tile_pool(name="ps_val", bufs=2, space="PSUM") as psV,  # [128,512] = 1 bank
        ):
            # ---------- constants / weights ----------
            w2b = {}
            for x in "qkv":
                stage = xtp.tile([P, NKT, F], FP32, name="xt")
                nc.sync.dma_start(stage[:], W2[x].ap().rearrange("(kt p) f -> p kt f", p=P))
                w2b[x] = persist.tile([P, NKT, F], BF16, name=f"w2b_{x}")
                nc.vector.tensor_copy(w2b[x][:], stage[:])
            b2q_sb = persist.tile([F, 1], FP32, name="b2q_sb")
            nc.sync.dma_start(b2q_sb[:], b2q[:])
            b2k_sb = persist.tile([F, 1], FP32, name="b2k_sb")
            nc.sync.dma_start(b2k_sb[:], b2k[:])
            bvb_sb = persist.tile([P, HC * (HD + 1)], FP32, name="bvb_sb")
            nc.sync.dma_start(bvb_sb[:], bvb[:])
            bpT_sb = persist.tile([P, D // P], FP32, name="bpT_sb")
            nc.sync.dma_start(bpT_sb[:], bpT[:])
            wpTb = persist.tile([P, NKT, D], BF16, name="wpTb")

            def load_wpT(half):
                stage = xtp.tile([P, NKT, D // 2], FP32, name="xt")
                nc.sync.dma_start(
                    stage[:],
                    WpT.ap().rearrange("(kt p) o -> p kt o", p=P)[:, :, half * 512:(half + 1) * 512],
                )
                nc.vector.tensor_copy(wpTb[:, :, half * 512:(half + 1) * 512], stage[:])
            # head selector: oh2[i, m] = (m // 64 == i); broadcasts [2,512] recips to [128,512]
            onehot2 = persist.tile([HC, P], FP32, name="onehot2")
            nc.sync.dma_start(onehot2[:], onehot_d[:])

            # warmup collective: absorb ncfw first-trigger latency while DMA streams
            warm_in = nc.dram_tensor("cc_warm_in", [M, 1, 64], BF16)
            warm_out = nc.dram_tensor("cc_warm_out", [M, 1, 64], BF16)
            wtile = small.tile([1, M * 64], BF16, name="wtile")
            nc.vector.memset(wtile[:], 1.0)
            nc.gpsimd.dma_start(warm_in.ap().rearrange("j p r -> p (j r)"), wtile[:])
            nc.gpsimd.collective_compute(
                "AllToAll", ALU.bypass, replica_groups=[list(range(M))],
                ins=[warm_in[:]], outs=[warm_out[:]])

            # persistent activations
            Qt = persist.tile([F, T], BF16, name="Qt")        # [2h*64, tok]
            Kt = persist.tile([F, T], BF16, name="Kt")
            Vx = persist.tile([P, T // P, HC * (HD + 1)], BF16, name="Vx")
            valT = persist.tile([F, T], BF16, name="valT")
            nc.vector.memset(Vx[:, :, HD:HD + 1], 1.0)        # ones columns
            nc.vector.memset(Vx[:, :, 2 * HD + 1:2 * HD + 2], 1.0)

            # ---------- phase A pieces (one 512-token chunk of one tensor) ----------
            def load_chunk(x, ch):
                t0 = ch * 512
                stage = xtp.tile([P, NKT, 512], FP32, name="xt")
                nc.sync.dma_start(
                    stage[:],
                    XT[x].ap().rearrange("(kt p) t -> p kt t", p=P)[:, :, t0:t0 + 512],
                )
                xtb = xtbp.tile([P, NKT, 512], BF16, name="xtb")
                nc.vector.tensor_copy(xtb[:], stage[:])
                return xtb

            def proj_qk(x, ch, dest, sc, bias, xtb=None):
                t0 = ch * 512
                if xtb is None:
                    xtb = load_chunk(x, ch)
                ps = psA.tile([P, 1024], FP32, name="ps_st")
                for kt in range(NKT):
                    nc.tensor.matmul(ps[:, 0:512], lhsT=w2b[x][:, kt, :], rhs=xtb[:, kt, :],
                                     start=(kt == 0), stop=(kt == NKT - 1))
                nc.vector.tensor_scalar(dest[:, t0:t0 + 512], ps[:, 0:512], sc, bias[:, 0:1],
                                        op0=ALU.mult, op1=ALU.add)

            def proj_v(ch, xtb=None):
                if xtb is None:
                    xtb = load_chunk("v", ch)
                vps = psA.tile([P, 1024], FP32, name="ps_st")
                for sub in range(4):
                    for kt in range(NKT):
                        nc.tensor.matmul(vps[:, sub * F:(sub + 1) * F],
                                         lhsT=xtb[:, kt, sub * P:(sub + 1) * P],
                                         rhs=w2b["v"][:, kt, :],
                                         start=(kt == 0), stop=(kt == NKT - 1))
                for sub in range(4):
                    tt = ch * 4 + sub
                    for h in range(HC):
                        nc.vector.tensor_add(Vx[:, tt, h * 65:h * 65 + HD],
                                             vps[:, sub * F + h * HD:sub * F + (h + 1) * HD],
                                             bvb_sb[:, h * 65:h * 65 + HD])

            # ---------- Phase B + incremental normalization ----------
            def phase_b(b, qc, extra_work=None):
                q0 = b * S + qc * 512
                vps = [psV.tile([P, 512], FP32, name="ps_val") for _ in range(HC)]
                est_prev = None
                for dk in range(NDK):
                    if extra_work is not None and dk in extra_work:
                        extra_work[dk]()
                    k0 = b * S + dk * 256
                    est = []
                    for h in range(HC):
                        fo = h * HD
                        stp = psA.tile([P, 1024], FP32, name="ps_st")
                        for half in range(2):
                            nc.tensor.matmul(stp[:, half * 512:(half + 1) * 512],
                                             lhsT=Kt[fo:fo + HD, k0 + half * P:k0 + (half + 1) * P],
                                             rhs=Qt[fo:fo + HD, q0:q0 + 512],
                                             start=True, stop=True)
                        e = estp.tile([P, 1024], BF16, name="est")
                        nc.scalar.activation(e[:], stp[:], AF.Exp)
                        est.append(e)
                    # val MMs for the previous double-tile (keeps PE fed while ACT runs)
                    if est_prev is not None:
                        emit_val(b, qc, dk - 1, est_prev, vps)
                    est_prev = est
                emit_val(b, qc, NDK - 1, est_prev, vps)
                # denominators -> den tile rows [2, 512]
                den_t = denp.tile([HC, 512], FP32, name="den")
                for h in range(HC):
                    dstage = small.tile([P, 512], FP32, name="dstage")
                    nc.vector.tensor_copy(dstage[HD:HD + 1, :], vps[h][HD:HD + 1, :])
                    den_eng = nc.gpsimd if b == 0 else nc.sync
                    den_eng.dma_start(den_t[h:h + 1, :], dstage[HD:HD + 1, :])
                    # unnormalized val^T -> valT (bf16)
                    nc.vector.tensor_copy(valT[h * HD:(h + 1) * HD, q0:q0 + 512],
                                          vps[h][0:HD, :])
                rcp_t = denp.tile([HC, 512], FP32, name="rcp")
                nc.vector.reciprocal_approx_fast(rcp_t[:], den_t[:])
                rbp = psV.tile([P, 512], FP32, name="ps_val")
                nc.tensor.matmul(rbp[:], lhsT=onehot2[:], rhs=rcp_t[:], start=True, stop=True)
                nc.vector.tensor_mul(valT[:, q0:q0 + 512], rbp[:], valT[:, q0:q0 + 512])

            def emit_val(b, qc, dk, est, vps):
                for h in range(HC):
                    for half in range(2):
                        kt = dk * 2 + half
                        nc.tensor.matmul(vps[h][0:HD + 1, :],
                                         lhsT=Vx[:, b * NST + kt, h * 65:(h + 1) * 65],
                                         rhs=est[h][:, half * 512:(half + 1) * 512],
                                         start=(kt == 0), stop=(kt == NST - 1))

            def a2a_send(b):
                nc.gpsimd.dma_start(
                    a2a_in[b].ap().rearrange("j p r -> p j r"),
                    valT.rearrange("p (bb j r) -> p bb j r", bb=B, j=M, r=SR)[:, b, :, :],
                )
                nc.gpsimd.collective_compute(
                    "AllToAll", ALU.bypass,
                    replica_groups=[list(range(M))],
                    ins=[a2a_in[b][:]], outs=[a2a_out[b][:]],
                )

            def phase_d_og(b, concatT, og):
                    ops = psA.tile([P, 1024], FP32, name="ps_st")
                    for sub in range(4):
                        oc = og * 4 + sub
                        for ft in range(NKT):
                            nc.tensor.matmul(ops[:, sub * SR:(sub + 1) * SR],
                                             lhsT=wpTb[:, ft, oc * P:(oc + 1) * P],
                                             rhs=concatT[:, ft, :],
                                             start=(ft == 0), stop=(ft == NKT - 1))
                    ot = small.tile([P, 1024], FP32, name="ot")
                    for sub in range(4):
                        oc = og * 4 + sub
                        nc.vector.tensor_scalar_add(ot[:, sub * SR:(sub + 1) * SR],
                                                    ops[:, sub * SR:(sub + 1) * SR],
                                                    bpT_sb[:, oc:oc + 1])
                    nc.gpsimd.dma_start(
                        outT.ap().rearrange("(og oc p) (bb r) -> p og oc bb r", p=P, oc=4, bb=B)[:, og, :, b, :],
                        ot.rearrange("p (oc r) -> p oc r", oc=4))

            def phase_d(b, concatT):
                nc.sync.dma_start(concatT[:], a2a_out[b].ap().rearrange("j p r -> p j r"))
                for og in range(2):
                    ops = psA.tile([P, 1024], FP32, name="ps_st")
                    for sub in range(4):
                        oc = og * 4 + sub
                        for ft in range(NKT):
                            nc.tensor.matmul(ops[:, sub * SR:(sub + 1) * SR],
                                             lhsT=wpTb[:, ft, oc * P:(oc + 1) * P],
                                             rhs=concatT[:, ft, :],
                                             start=(ft == 0), stop=(ft == NKT - 1))
                    ot = small.tile([P, 1024], FP32, name="ot")
                    for sub in range(4):
                        oc = og * 4 + sub
                        nc.vector.tensor_scalar_add(ot[:, sub * SR:(sub + 1) * SR],
                                                    ops[:, sub * SR:(sub + 1) * SR],
                                                    bpT_sb[:, oc:oc + 1])
                    nc.gpsimd.dma_start(
                        outT.ap().rearrange("(og oc p) (bb r) -> p og oc bb r", p=P, oc=4, bb=B)[:, og, :, b, :],
                        ot.rearrange("p (oc r) -> p oc r", oc=4))

            # ---------- emission ----------
            # Attention starts after only k0+v0+q0; remaining projections are
            # interleaved into phase-B chunks as extra work, with their loads
            # pre-issued in consumption order so PE never stalls on a cast.
            concatT0 = persist.tile([P, M, SR], BF16, name="concatT0")
            concatT1 = persist.tile([P, M, SR], BF16, name="concatT1")
            pre = {}

            def pk(x, ch):
                pre[f"{x}{ch}"] = load_chunk(x, ch)

            def pj(x, ch):
                if x == "v":
                    proj_v(ch, xtb=pre[f"v{ch}"])
                elif x == "k":
                    proj_qk("k", ch, Kt, 1.0, b2k_sb, xtb=pre[f"k{ch}"])
                else:
                    proj_qk("q", ch, Qt, SCALE, b2q_sb, xtb=pre[f"q{ch}"])

            proj_qk("k", 0, Kt, 1.0, b2k_sb)
            proj_v(0)
            proj_qk("q", 0, Qt, SCALE, b2q_sb)
            pk("k", 1); pk("v", 1); pk("k", 2); pk("v", 2)
            phase_b(0, 0, extra_work={
                2: lambda: (pj("k", 1), pj("v", 1), pk("k", 3), pk("v", 3)),
                4: lambda: (pj("k", 2), pj("v", 2), pk("q", 1), pk("k", 4)),
                6: lambda: (pj("k", 3), pj("v", 3), pk("v", 4)),
            })
            pj("q", 1)
            phase_b(0, 1, extra_work={
                3: lambda: (pj("k", 4), pj("v", 4), pk("q", 2), pk("k", 5), pk("v", 5)),
            })
            load_wpT(0)
            pj("q", 2)
            phase_b(0, 2, extra_work={
                3: lambda: (pj("k", 5), pj("v", 5), pk("q", 3), pk("k", 6), pk("v", 6)),
            })
            load_wpT(1)
            pj("q", 3)
            phase_b(0, 3, extra_work={
                2: lambda: (pj("k", 6), pj("v", 6), pk("k", 7), pk("v", 7)),
                5: lambda: (pj("k", 7), pj("v", 7), pk("q", 4)),
            })
            a2a_send(0)
            pj("q", 4)
            pk("q", 5)
            phase_b(1, 0, extra_work={4: lambda: (pj("q", 5), pk("q", 6))})
            phase_b(1, 1, extra_work={
                2: lambda: nc.sync.dma_start(
                    concatT0[:], a2a_out[0].ap().rearrange("j p r -> p j r")),
                4: lambda: (pj("q", 6), pk("q", 7)),
            })
            phase_b(1, 2, extra_work={4: lambda: pj("q", 7)})
            phase_d_og(0, concatT0, 0)
            phase_b(1, 3)
            a2a_send(1)
            phase_d_og(0, concatT0, 1)
            # idempotent re-issue: keeps PE busy through the a2a(1) flight so
            # HAM stays warm for phase D(1) (rewrites identical outT bytes)
            phase_d_og(0, concatT0, 0)
            nc.sync.dma_start(concatT1[:], a2a_out[1].ap().rearrange("j p r -> p j r"))
            phase_d_og(1, concatT1, 0)
            phase_d_og(1, concatT1, 1)

    nc.compile()
    return nc


def _host_prep(inputs):
    f32 = np.float32
    QT = np.ascontiguousarray(inputs["Q_in"].reshape(T, D).T).astype(f32, copy=False)
    KT = np.ascontiguousarray(inputs["K_in"].reshape(T, D).T).astype(f32, copy=False)
    VT = np.ascontiguousarray(inputs["V_in"].reshape(T, D).T).astype(f32, copy=False)
    WpT = np.ascontiguousarray(inputs["Wp"].T).astype(f32, copy=False)
    bpT = np.ascontiguousarray(inputs["bp"].reshape(D // P, P).T).astype(f32, copy=False)
    oh2 = np.zeros((HC, P), f32)
    for h in range(HC):
        oh2[h, h * HD:(h + 1) * HD] = 1.0
    in_maps = []
    for c in range(M):
        sl = slice(c * HC, (c + 1) * HC)
        m = {
            "qT": QT, "kT": KT, "vT": VT, "WpT": WpT, "bpT": bpT, "onehot": oh2,
            "w2q": np.ascontiguousarray(inputs["Wq"][sl].transpose(1, 0, 2).reshape(D, F)).astype(f32, copy=False),
            "w2k": np.ascontiguousarray(inputs["Wk"][sl].transpose(1, 0, 2).reshape(D, F)).astype(f32, copy=False),
            "w2v": np.ascontiguousarray(inputs["Wv"][sl].transpose(1, 0, 2).reshape(D, F)).astype(f32, copy=False),
            "b2q": (inputs["bq"][sl].reshape(F, 1) * SCALE).astype(f32),
            "b2k": inputs["bk"][sl].reshape(F, 1).astype(f32),
        }
        bvb = np.zeros((P, HC * (HD + 1)), f32)
        for h in range(HC):
            bvb[:, h * 65:h * 65 + HD] = inputs["bv"][c * HC + h][None, :]
        m["bvb"] = bvb
        in_maps.append(m)
    return in_maps


_LAST = {"exec_time_ns": None}


def kernel(**inputs):
    inputs = {k: np.asarray(v) for k, v in inputs.items()}
    if "nc" not in _CACHE:
        _CACHE["nc"] = build()
    nc = _CACHE["nc"]
    in_maps = _host_prep(inputs)
    res = run_bass_kernel_spmd(nc, in_maps, core_ids=list(range(M)),
                               trace=_LAST.get("trace", False))
    _LAST["exec_time_ns"] = res.exec_time_ns
    _LAST["res"] = res
    out = np.zeros((T, D), np.float32)
    for c in range(M):
        oT = res.results[c]["outT"]  # [D, TS] = [D, (b sr)]
        for b in range(B):
            out[b * S + c * SR:b * S + (c + 1) * SR, :] = oT[:, b * SR:(b + 1) * SR].T
    return out.reshape(B, S, D)
